# revision 10
# baseline (speedup 1.0000x reference)
"""BertSelfAttention on 8 TRN2 NeuronCores (Bass/Tile).

Sharding: core = (b, g) for b in 0..3 (batch), g in 0..1 (head group of 6
heads = 384 cols of the QKV projections). Pure SPMD, no collectives.

Per-core device kernel (all bf16 storage, f32 PSUM):
  xT  [768, 2048]  = hidden_states[b].T          (host pre-transpose)
  wq/wk/wv [768, 384] = W[g-slice].T             (kxm layout)
  QT/KT = w.T @ xT -> [384, 2048] (d on partitions) + bias via DVE
  V = xT.T @ wv -> [2048, 384] (s on partitions), stored per 128-row chunk
  as [128, 6, 65] with a ones column per head (softmax denominator).

  Attention per (head, 1024-q block) "unit":
   phase 1, per kc (16 key chunks of 128):
     scores^T [128k, 1024q] = KT_h[:, kc].T @ QT_h on PE (2 matmuls)
     expS = exp(scores/8 + mask - 2) -> bf16 SBUF, on EITHER:
       - ACT: native Exp (scale=0.125, per-kc bias AP = mask-2), or
       - DVE (offload, tunable subset): two Schraudolph bf16 bit-tricks
         offset by half an octave, combined e = t2*sqrt(2) + t1 via
         scalar_tensor_tensor (max rel err ~0.9%). uint16 convert on DVE
         saturates+rounds-to-nearest (verified on HW).
     The -2 shift cancels between numerator and denominator.
   phase 2 (emitted interleaved into the NEXT unit's phase 1 so the
   in-order PE queue never serializes behind ACT), per qch (8 q chunks
   of 128):
     ctx[128q, 65] = sum_kc expS_kc[:, qch].T @ V_kc[:, h]  (16 matmuls,
     free dim 65 -> 27ns each; "flipped" orientation cuts ctx PE time 2x
     vs ctx^T since all 128 output partitions are used)
     row sums land in col 64 via the V ones column; DVE copies to SBUF,
     DMA to outT[q, h*65:...].

  The V projection and the m>=1 Q/K projection chunks are deferred and
  drained one item per (kc) slot into the attention stream to fill PE
  while ACT/DVE grind exps.

Host divides ctx cols 0..63 by col 64 (denominator), adds bv.
"""

import sys

for _p in ("/opt/trn_rl_repo",):
    if _p not in sys.path:
        sys.path.insert(0, _p)

import numpy as np
import ml_dtypes

import concourse.bass as bass  # noqa: F401
import concourse.mybir as mybir
from concourse import bacc, tile
from concourse.bass_utils import run_bass_kernel_spmd

AFT = mybir.ActivationFunctionType
ALU = mybir.AluOpType
BF16 = mybir.dt.bfloat16
F32 = mybir.dt.float32
U16 = mybir.dt.uint16

B, S, H = 4, 2048, 768
NH, HD = 12, 64
N_CORES = 8
NH_LOC = 6          # heads per core
DL = NH_LOC * HD    # 384 local projection cols
KT = H // 128       # 6 k-tiles over the hidden dim
M3 = DL // 128      # 3 m-chunks of the local projections
KC = S // 128       # 16 key chunks
QC = S // 1024      # 2 query blocks of 1024 per head
QCH = 8             # 128-row q chunks per block
HDV = HD + 1        # per-head V cols incl. ones column

# exp bit-trick constants: bf16 Schraudolph sample t1 = u16(scores*AT
# + bias); second sample half an octave lower derived by u16 subtract
# (4x DVE mode); combined with a bf16 add (2x mode): e = t1 + (t1-64
# bytes) = 0.85355*exp(arg) (constant scale; ACT chunks are scaled to
# match via LNS in their bias so the softmax ratio is unaffected).
AT = 23.083120625
TB1 = 15752.170070
AE = 184.664965     # byte16 per unit of exp argument (for mask folding)
LNS = -0.15834718   # ln((1+2**-0.5)/2)

# which (unit, kc) chunks run exp on DVE instead of ACT: none in the
# PE-bound unit 0; more in late units once V/QK copies are done and DVE
# has slack.
def dve_kcs(u):
    if u == 0:
        return ()
    if u >= 8:
        return (2, 5, 8, 11, 14)
    return (2, 6, 10, 14)

_CACHED = None


def _build():
    nc = bacc.Bacc("TRN2", target_bir_lowering=False, debug=False,
                   num_devices=N_CORES)
    # inputs are host-prearranged images of the SBUF tiles:
    # xH [128, k*2048]; wq/wk m-major [128, (m*6+k)*128]; wv k-major
    # [128, k*384]
    xH = nc.dram_tensor("xH", [128, KT * S], BF16, kind="ExternalInput").ap()
    wqH = nc.dram_tensor("wqH", [128, KT * DL], BF16,
                         kind="ExternalInput").ap()
    wkH = nc.dram_tensor("wkH", [128, KT * DL], BF16,
                         kind="ExternalInput").ap()
    wvH = nc.dram_tensor("wvH", [128, KT * DL], BF16,
                         kind="ExternalInput").ap()
    # packed small inputs: actb 0:16, dvb1 16:32, bq 32:35, bk 35:38
    bpackT = nc.dram_tensor("bpackT", [128, 2 * KC + 2 * M3], F32,
                            kind="ExternalInput").ap()
    outT = nc.dram_tensor("outT", [S, NH_LOC * HDV], F32,
                          kind="ExternalOutput").ap()

    with tile.TileContext(nc) as tc:
        with (
            tc.tile_pool(name="persist", bufs=1) as pp,
            tc.tile_pool(name="work", bufs=1) as wp,
            tc.tile_pool(name="psum", bufs=1, space="PSUM") as psp,
        ):
            # ---- persistent SBUF tensors (k-tiles packed side by side
            # in one tile each so input DMAs are few and large) ----
            xall = pp.tile([128, KT * S], BF16, tag="xall", name="xall")
            wqall = pp.tile([128, KT * DL], BF16, tag="wqall", name="wqall")
            wkall = pp.tile([128, KT * DL], BF16, tag="wkall", name="wkall")
            wvall = pp.tile([128, KT * DL], BF16, tag="wvall", name="wvall")
            x_t = [xall[:, k * S:(k + 1) * S] for k in range(KT)]
            # wq/wk are m-major: slice (m, k) at (m*KT+k)*128
            wv_t = [wvall[:, k * DL:(k + 1) * DL] for k in range(KT)]

            def wqk(wall, k, m):
                c = (m * KT + k) * 128
                return wall[:, c:c + 128]
            qT_t = [pp.tile([128, S], BF16, tag=f"qT{m}", name=f"qT{m}")
                    for m in range(M3)]
            kT_t = [pp.tile([128, S], BF16, tag=f"kT{m}", name=f"kT{m}")
                    for m in range(M3)]
            v_t = [pp.tile([128, NH_LOC * HDV], BF16, tag=f"v{s}",
                           name=f"v{s}") for s in range(KC)]
            bpack = pp.tile([128, 2 * KC + 2 * M3], F32, tag="bpack",
                            name="bpack")
            actb = bpack[:, 0:KC]
            dvb1 = bpack[:, KC:2 * KC]
            bq_sb = bpack[:, 2 * KC:2 * KC + M3]
            bk_sb = bpack[:, 2 * KC + M3:2 * KC + 2 * M3]
            ones3 = pp.tile([128, 3], BF16, tag="ones3", name="ones3")
            nc.vector.memset(ones3[:], 1.0)

            # ---- DMA inputs: sources mirror SBUF layouts, so every DMA
            # is a contiguous block copy; ordered so the m=0 projection and
            # unit-0 attention start earliest.
            xv = xall.rearrange("p (k c) -> p k c", c=S)
            xsv = xH.rearrange("p (k c) -> p k c", c=S)
            nc.sync.dma_start(xv[:, :, 0:512], xsv[:, :, 0:512])
            nc.sync.dma_start(wqall[:, 0:6 * 128], wqH[:, 0:6 * 128])
            nc.sync.dma_start(wkall[:, 0:6 * 128], wkH[:, 0:6 * 128])
            nc.sync.dma_start(bpack[:], bpackT[:])
            nc.sync.dma_start(xv[:, :, 512:S], xsv[:, :, 512:S])
            nc.sync.dma_start(wvall[:], wvH[:])
            nc.sync.dma_start(wqall[:, 6 * 128:], wqH[:, 6 * 128:])
            nc.sync.dma_start(wkall[:, 6 * 128:], wkH[:, 6 * 128:])

            def proj_one(dst, w_t, b_sb, m, n, width=512):  # w_t: wall AP
                """Project one s-chunk of QT[m] or KT[m]."""
                ns = slice(n * width, (n + 1) * width)
                ps = psp.tile([128, width], F32, tag="psS", bufs=3,
                              name=f"psp{dst[m].name}_{n}_{width}")
                for k in range(KT):
                    nc.tensor.matmul(
                        ps[:],
                        wqk(w_t, k, m),
                        x_t[k][:, ns],
                        start=(k == 0), stop=(k == KT - 1))
                nc.vector.tensor_scalar_add(
                    dst[m][:, ns], ps[:], b_sb[:, m:m + 1])

            def proj_v(sc, ha, hb):
                """Project heads [ha, hb) of one 128-row s-chunk of V."""
                nh = hb - ha
                ps = psp.tile([128, nh * HD], F32, tag="psS", bufs=3,
                              name=f"psv{sc}_{ha}")
                for k in range(KT):
                    nc.tensor.matmul(
                        ps[:],
                        x_t[k][:, sc * 128:(sc + 1) * 128],
                        wv_t[k][:, ha * HD:hb * HD],
                        start=(k == 0), stop=(k == KT - 1))
                v3 = v_t[sc].rearrange("p (h e) -> p h e", e=HDV)
                nc.vector.tensor_copy(
                    v3[:, ha:hb, 0:HD],
                    ps[:].rearrange("p (h e) -> p h e", e=HD))
                nc.gpsimd.tensor_copy(v3[:, ha:hb, HD:HDV], ones3[:, 0:nh])

            # Deferred projection work, drained into the attention stream.
            # Deadlines (strict-before): m1 before unit 4 phase1, V heads 3-6
            # before unit 6 phase2 (emitted during unit 7), m2 before unit 8.
            pending = []
            for n in range(8):
                pending.append(("k", 1, n))
                pending.append(("q", 1, n))
            pending += [("v", sc, 3, 6) for sc in range(KC)]
            for n in range(8):
                pending.append(("k", 2, n))
                pending.append(("q", 2, n))

            def drain_one():
                if pending:
                    item = pending.pop(0)
                    if item[0] == "v":
                        proj_v(item[1], item[2], item[3])
                    elif item[0] == "q":
                        proj_one(qT_t, wqall, bq_sb, item[1], item[2], 256)
                    else:
                        proj_one(kT_t, wkall, bk_sb, item[1], item[2], 256)

            # m=0 Q/K projections for q-columns 0-1024 up front; n=2,3 are
            # emitted early in unit 0 (needed by unit 1 = h0, qc1).
            for n in range(2):
                proj_one(qT_t, wqall, bq_sb, 0, n)
                proj_one(kT_t, wkall, bk_sb, 0, n)

            # ---- attention units ----
            units = [(h, qc) for h in range(NH_LOC) for qc in range(QC)]
            exp_tiles = {}

            osb_tiles = {}

            def emit_ctx_chain(u, qch):
                """ctx[128q, 65] for unit u, q chunk qch (flip orientation),
                then DVE copy into the unit's staging tile; one DMA per
                unit after the last chain (HWDGE overhead is per-DMA)."""
                h, qc = units[u]
                ctx = psp.tile([128, 512], F32, tag="ctx", bufs=2,
                               name=f"ctx{u}_{qch}")
                qs = slice(qch * 128, (qch + 1) * 128)
                for kc in range(KC):
                    nc.tensor.matmul(
                        ctx[:, 0:HDV],
                        exp_tiles[(u, kc)][:, qs],
                        v_t[kc][:, h * HDV:(h + 1) * HDV],
                        start=(kc == 0), stop=(kc == KC - 1))
                if qch == 0:
                    osb_tiles[u] = wp.tile([128, QCH * HDV], F32, tag="osb",
                                           bufs=2, name=f"osb{u}")
                osb = osb_tiles[u]
                nc.vector.tensor_copy(
                    osb[:, qch * HDV:(qch + 1) * HDV], ctx[:, 0:HDV])
                if qch == QCH - 1:
                    q0 = qc * 1024
                    dst = outT[q0:q0 + 1024, h * HDV:(h + 1) * HDV]
                    nc.sync.dma_start(
                        dst.rearrange("(a p) c -> p a c", p=128),
                        osb[:].rearrange("p (a c) -> p a c", c=HDV))

            for u, (h, qc) in enumerate(units):
                m, off = divmod(h, 2)
                off *= HD
                kTh = kT_t[m][off:off + HD, :]
                qTh = qT_t[m][off:off + HD, :]
                q0 = qc * 1024
                for kc in range(KC):
                    ks = slice(kc * 128, (kc + 1) * 128)
                    psS = psp.tile([128, 1024], F32, tag="psS", bufs=3,
                                   name=f"psS{u}_{kc}")
                    for qq in range(2):
                        nc.tensor.matmul(
                            psS[:, qq * 512:(qq + 1) * 512],
                            kTh[:, ks],
                            qTh[:, q0 + qq * 512:q0 + (qq + 1) * 512],
                            start=True, stop=True)
                    expS = wp.tile([128, 1024], BF16, tag="expS", bufs=32,
                                   name=f"expS{u}_{kc}")
                    exp_tiles[(u, kc)] = expS
                    if kc in dve_kcs(u):
                        t1 = wp.tile([128, 1024], BF16, tag="tk1", bufs=4,
                                     name=f"tk1_{u}_{kc}")
                        t2 = wp.tile([128, 1024], BF16, tag="tk2", bufs=4,
                                     name=f"tk2_{u}_{kc}")
                        nc.vector.tensor_scalar(
                            t1[:].bitcast(U16), psS[:], AT,
                            dvb1[:, kc:kc + 1], op0=ALU.mult, op1=ALU.add)
                        nc.vector.tensor_scalar(
                            t2[:].bitcast(U16), t1[:].bitcast(U16), 64, None,
                            op0=ALU.subtract)
                        nc.vector.tensor_tensor(
                            expS[:], t1[:], t2[:], op=ALU.add)
                    else:
                        nc.scalar.activation(expS[:], psS[:], AFT.Exp,
                                             bias=actb[:, kc:kc + 1],
                                             scale=0.125)
                    # interleave: previous unit's ctx chains (one per 2 kc)
                    if u > 0 and kc % 2 == 1:
                        emit_ctx_chain(u - 1, kc // 2)
                    # deferred projection drains (kc%3 cadence: ~6/unit,
                    # lasts through unit 8 to keep PE fed in late units)
                    if u == 0:
                        proj_v(kc, 0, 3)
                        if kc in (4, 5):
                            proj_one(qT_t, wqall, bq_sb, 0, kc - 4 + 2)
                        elif kc in (6, 7):
                            proj_one(kT_t, wkall, bk_sb, 0, kc - 6 + 2)
                    elif kc % 3 == 0:
                        drain_one()
                        if u >= 7 and pending and kc == 9:
                            drain_one()
                if u == len(units) - 1:
                    for qch in range(QCH):
                        emit_ctx_chain(u, qch)

    nc.compile()
    return nc


def _get_nc():
    global _CACHED
    if _CACHED is None:
        _CACHED = _build()
    return _CACHED


def kernel(hidden_states, attention_mask, Wq, bq, Wk, bk, Wv, bv):
    hidden_states = np.asarray(hidden_states, np.float32)
    attention_mask = np.asarray(attention_mask, np.float32)
    Wq, Wk, Wv = (np.asarray(w, np.float32) for w in (Wq, Wk, Wv))
    bq, bk, bv = (np.asarray(b, np.float32) for b in (bq, bk, bv))

    nc = _get_nc()
    in_maps = []
    for core in range(N_CORES):
        b, g = divmod(core, 2)
        cs = slice(g * DL, (g + 1) * DL)
        xTb = hidden_states[b].T.reshape(KT, 128, S).transpose(1, 0, 2)
        xTb = np.ascontiguousarray(xTb).reshape(128, KT * S).astype(
            ml_dtypes.bfloat16)
        mk = np.ascontiguousarray(
            attention_mask[b, 0, 0, :].reshape(KC, 128).T).astype(np.float32)
        bp = np.concatenate([
            mk - 2.0 + LNS,
            AE * mk + TB1,
            bq[cs].reshape(M3, 128).T,
            bk[cs].reshape(M3, 128).T,
        ], axis=1).astype(np.float32)
        def mmaj(W):   # [768, 384] -> [128, (m*6+k)*128] m-major image
            a = W[cs, :].T.reshape(KT, 128, M3, 128).transpose(1, 2, 0, 3)
            return np.ascontiguousarray(a).reshape(
                128, KT * DL).astype(ml_dtypes.bfloat16)

        def kmaj(W):   # [768, 384] -> [128, k*384] k-major image
            a = W[cs, :].T.reshape(KT, 128, DL).transpose(1, 0, 2)
            return np.ascontiguousarray(a).reshape(
                128, KT * DL).astype(ml_dtypes.bfloat16)

        in_maps.append({
            "xH": xTb,
            "wqH": mmaj(Wq),
            "wkH": mmaj(Wk),
            "wvH": kmaj(Wv),
            "bpackT": np.ascontiguousarray(bp),
        })

    res = run_bass_kernel_spmd(nc, in_maps, core_ids=list(range(N_CORES)))

    out = np.empty((B, S, H), np.float32)
    for core in range(N_CORES):
        b, g = divmod(core, 2)
        oT = res.results[core]["outT"]              # [2048, 6*65]
        oT = oT.reshape(S, NH_LOC, HDV)
        ctx = oT[:, :, :HD] / oT[:, :, HD:HDV]      # [2048, 6, 64]
        cols = slice(g * DL, (g + 1) * DL)
        out[b, :, cols] = ctx.reshape(S, DL) + bv[cols][None, :]
    return out


# revision 11
# speedup vs baseline: 1.0127x; 1.0127x over previous
"""BertSelfAttention on 8 TRN2 NeuronCores (Bass/Tile).

Sharding: core = (b, g) for b in 0..3 (batch), g in 0..1 (head group of 6
heads = 384 cols of the QKV projections). Pure SPMD, no collectives.

Per-core device kernel (all bf16 storage, f32 PSUM):
  xT  [768, 2048]  = hidden_states[b].T          (host pre-transpose)
  wq/wk/wv [768, 384] = W[g-slice].T             (kxm layout)
  QT/KT = w.T @ xT -> [384, 2048] (d on partitions) + bias via DVE
  V = xT.T @ wv -> [2048, 384] (s on partitions), stored per 128-row chunk
  as [128, 6, 65] with a ones column per head (softmax denominator).

  Attention per (head, 1024-q block) "unit":
   phase 1, per kc (16 key chunks of 128):
     scores^T [128k, 1024q] = KT_h[:, kc].T @ QT_h on PE (2 matmuls)
     expS = exp(scores/8 + mask - 2) -> bf16 SBUF, on EITHER:
       - ACT: native Exp (scale=0.125, per-kc bias AP = mask-2), or
       - DVE (offload, tunable subset): two Schraudolph bf16 bit-tricks
         offset by half an octave, combined e = t2*sqrt(2) + t1 via
         scalar_tensor_tensor (max rel err ~0.9%). uint16 convert on DVE
         saturates+rounds-to-nearest (verified on HW).
     The -2 shift cancels between numerator and denominator.
   phase 2 (emitted interleaved into the NEXT unit's phase 1 so the
   in-order PE queue never serializes behind ACT), per qch (8 q chunks
   of 128):
     ctx[128q, 65] = sum_kc expS_kc[:, qch].T @ V_kc[:, h]  (16 matmuls,
     free dim 65 -> 27ns each; "flipped" orientation cuts ctx PE time 2x
     vs ctx^T since all 128 output partitions are used)
     row sums land in col 64 via the V ones column; DVE copies to SBUF,
     DMA to outT[q, h*65:...].

  The V projection and the m>=1 Q/K projection chunks are deferred and
  drained one item per (kc) slot into the attention stream to fill PE
  while ACT/DVE grind exps.

Host divides ctx cols 0..63 by col 64 (denominator), adds bv.
"""

import sys

for _p in ("/opt/trn_rl_repo",):
    if _p not in sys.path:
        sys.path.insert(0, _p)

import numpy as np
import ml_dtypes

import concourse.bass as bass  # noqa: F401
import concourse.mybir as mybir
from concourse import bacc, tile
from concourse.bass_utils import run_bass_kernel_spmd

AFT = mybir.ActivationFunctionType
ALU = mybir.AluOpType
BF16 = mybir.dt.bfloat16
F32 = mybir.dt.float32
U16 = mybir.dt.uint16

B, S, H = 4, 2048, 768
NH, HD = 12, 64
N_CORES = 8
NH_LOC = 6          # heads per core
DL = NH_LOC * HD    # 384 local projection cols
KT = H // 128       # 6 k-tiles over the hidden dim
M3 = DL // 128      # 3 m-chunks of the local projections
KC = S // 128       # 16 key chunks
QC = S // 1024      # 2 query blocks of 1024 per head
QCH = 8             # 128-row q chunks per block
HDV = HD + 1        # per-head V cols incl. ones column

# exp bit-trick constants: bf16 Schraudolph sample t1 = u16(scores*AT
# + bias); second sample half an octave lower derived by u16 subtract
# (4x DVE mode); combined with a bf16 add (2x mode): e = t1 + (t1-64
# bytes) = 0.85355*exp(arg) (constant scale; ACT chunks are scaled to
# match via LNS in their bias so the softmax ratio is unaffected).
AT = 23.083120625
TB1 = 15752.170070
AE = 184.664965     # byte16 per unit of exp argument (for mask folding)
LNS = -0.15834718   # ln((1+2**-0.5)/2)

# which (unit, kc) chunks run exp on DVE instead of ACT: none in the
# PE-bound unit 0; more in late units once V/QK copies are done and DVE
# has slack.
def dve_kcs(u):
    if u == 0:
        return ()
    if u <= 2:
        return (6, 14)
    if u >= 8:
        return (2, 5, 8, 11, 14)
    return (2, 6, 10, 14)

_CACHED = None


def _build():
    nc = bacc.Bacc("TRN2", target_bir_lowering=False, debug=False,
                   num_devices=N_CORES)
    # inputs are host-prearranged images of the SBUF tiles:
    # xH [128, k*2048]; wq/wk m-major [128, (m*6+k)*128]; wv k-major
    # [128, k*384]
    xH = nc.dram_tensor("xH", [128, KT * S], BF16, kind="ExternalInput").ap()
    wqH = nc.dram_tensor("wqH", [128, KT * DL], BF16,
                         kind="ExternalInput").ap()
    wkH = nc.dram_tensor("wkH", [128, KT * DL], BF16,
                         kind="ExternalInput").ap()
    wvH = nc.dram_tensor("wvH", [128, KT * DL], BF16,
                         kind="ExternalInput").ap()
    # packed small inputs: actb 0:16, dvb1 16:32, bq 32:35, bk 35:38
    bpackT = nc.dram_tensor("bpackT", [128, 2 * KC + 2 * M3], F32,
                            kind="ExternalInput").ap()
    outT = nc.dram_tensor("outT", [S, NH_LOC * HDV], F32,
                          kind="ExternalOutput").ap()

    with tile.TileContext(nc) as tc:
        with (
            tc.tile_pool(name="persist", bufs=1) as pp,
            tc.tile_pool(name="work", bufs=1) as wp,
            tc.tile_pool(name="psum", bufs=1, space="PSUM") as psp,
        ):
            # ---- persistent SBUF tensors (k-tiles packed side by side
            # in one tile each so input DMAs are few and large) ----
            xall = pp.tile([128, KT * S], BF16, tag="xall", name="xall")
            wqall = pp.tile([128, KT * DL], BF16, tag="wqall", name="wqall")
            wkall = pp.tile([128, KT * DL], BF16, tag="wkall", name="wkall")
            wvall = pp.tile([128, KT * DL], BF16, tag="wvall", name="wvall")
            x_t = [xall[:, k * S:(k + 1) * S] for k in range(KT)]
            # wq/wk are m-major: slice (m, k) at (m*KT+k)*128
            wv_t = [wvall[:, k * DL:(k + 1) * DL] for k in range(KT)]

            def wqk(wall, k, m):
                c = (m * KT + k) * 128
                return wall[:, c:c + 128]
            qT_t = [pp.tile([128, S], BF16, tag=f"qT{m}", name=f"qT{m}")
                    for m in range(M3)]
            kT_t = [pp.tile([128, S], BF16, tag=f"kT{m}", name=f"kT{m}")
                    for m in range(M3)]
            v_t = [pp.tile([128, NH_LOC * HDV], BF16, tag=f"v{s}",
                           name=f"v{s}") for s in range(KC)]
            bpack = pp.tile([128, 2 * KC + 2 * M3], F32, tag="bpack",
                            name="bpack")
            actb = bpack[:, 0:KC]
            dvb1 = bpack[:, KC:2 * KC]
            bq_sb = bpack[:, 2 * KC:2 * KC + M3]
            bk_sb = bpack[:, 2 * KC + M3:2 * KC + 2 * M3]
            ones3 = pp.tile([128, 3], BF16, tag="ones3", name="ones3")
            nc.vector.memset(ones3[:], 1.0)

            # ---- DMA inputs: sources mirror SBUF layouts, so every DMA
            # is a contiguous block copy; ordered so the m=0 projection and
            # unit-0 attention start earliest.
            xv = xall.rearrange("p (k c) -> p k c", c=S)
            xsv = xH.rearrange("p (k c) -> p k c", c=S)
            nc.sync.dma_start(xv[:, :, 0:256], xsv[:, :, 0:256])
            nc.sync.dma_start(wqall[:, 0:6 * 128], wqH[:, 0:6 * 128])
            nc.sync.dma_start(wkall[:, 0:6 * 128], wkH[:, 0:6 * 128])
            nc.sync.dma_start(xv[:, :, 256:512], xsv[:, :, 256:512])
            nc.sync.dma_start(bpack[:], bpackT[:])
            nc.sync.dma_start(xv[:, :, 512:S], xsv[:, :, 512:S])
            nc.sync.dma_start(wvall[:], wvH[:])
            nc.sync.dma_start(wqall[:, 6 * 128:], wqH[:, 6 * 128:])
            nc.sync.dma_start(wkall[:, 6 * 128:], wkH[:, 6 * 128:])

            def proj_one(dst, w_t, b_sb, m, n, width=512):  # w_t: wall AP
                """Project one s-chunk of QT[m] or KT[m]."""
                ns = slice(n * width, (n + 1) * width)
                ps = psp.tile([128, width], F32, tag="psS", bufs=3,
                              name=f"psp{dst[m].name}_{n}_{width}")
                for k in range(KT):
                    nc.tensor.matmul(
                        ps[:],
                        wqk(w_t, k, m),
                        x_t[k][:, ns],
                        start=(k == 0), stop=(k == KT - 1))
                nc.vector.tensor_scalar_add(
                    dst[m][:, ns], ps[:], b_sb[:, m:m + 1])

            def proj_v(sc, ha, hb):
                """Project heads [ha, hb) of one 128-row s-chunk of V."""
                nh = hb - ha
                ps = psp.tile([128, nh * HD], F32, tag="psS", bufs=3,
                              name=f"psv{sc}_{ha}")
                for k in range(KT):
                    nc.tensor.matmul(
                        ps[:],
                        x_t[k][:, sc * 128:(sc + 1) * 128],
                        wv_t[k][:, ha * HD:hb * HD],
                        start=(k == 0), stop=(k == KT - 1))
                v3 = v_t[sc].rearrange("p (h e) -> p h e", e=HDV)
                nc.vector.tensor_copy(
                    v3[:, ha:hb, 0:HD],
                    ps[:].rearrange("p (h e) -> p h e", e=HD))
                nc.gpsimd.tensor_copy(v3[:, ha:hb, HD:HDV], ones3[:, 0:nh])

            # Deferred projection work, drained into the attention stream.
            # Deadlines (strict-before): m1 before unit 4 phase1, V heads 3-6
            # before unit 6 phase2 (emitted during unit 7), m2 before unit 8.
            pending = []
            for n in range(8):
                pending.append(("k", 1, n))
                pending.append(("q", 1, n))
            pending += [("v", sc, 3, 6) for sc in range(KC)]
            for n in range(8):
                pending.append(("k", 2, n))
                pending.append(("q", 2, n))

            def drain_one():
                if pending:
                    item = pending.pop(0)
                    if item[0] == "v":
                        proj_v(item[1], item[2], item[3])
                    elif item[0] == "q":
                        proj_one(qT_t, wqall, bq_sb, item[1], item[2], 256)
                    else:
                        proj_one(kT_t, wkall, bk_sb, item[1], item[2], 256)

            # m=0 Q/K projections for q-columns 0-1024 up front (first
            # chunks 256-wide so PE starts as soon as the first x DMA
            # lands); n=2,3 are emitted early in unit 0.
            proj_one(qT_t, wqall, bq_sb, 0, 0, 256)
            proj_one(kT_t, wkall, bk_sb, 0, 0, 256)
            proj_one(qT_t, wqall, bq_sb, 0, 1, 256)
            proj_one(kT_t, wkall, bk_sb, 0, 1, 256)
            proj_one(qT_t, wqall, bq_sb, 0, 2, 256)
            proj_one(kT_t, wkall, bk_sb, 0, 2, 256)
            proj_one(qT_t, wqall, bq_sb, 0, 3, 256)
            proj_one(kT_t, wkall, bk_sb, 0, 3, 256)

            # ---- attention units ----
            units = [(h, qc) for h in range(NH_LOC) for qc in range(QC)]
            exp_tiles = {}

            osb_tiles = {}

            def emit_ctx_chain(u, qch):
                """ctx[128q, 65] for unit u, q chunk qch (flip orientation),
                then DVE copy into the unit's staging tile; one DMA per
                unit after the last chain (HWDGE overhead is per-DMA)."""
                h, qc = units[u]
                ctx = psp.tile([128, 512], F32, tag="ctx", bufs=2,
                               name=f"ctx{u}_{qch}")
                qs = slice(qch * 128, (qch + 1) * 128)
                for kc in range(KC):
                    nc.tensor.matmul(
                        ctx[:, 0:HDV],
                        exp_tiles[(u, kc)][:, qs],
                        v_t[kc][:, h * HDV:(h + 1) * HDV],
                        start=(kc == 0), stop=(kc == KC - 1))
                if qch == 0:
                    osb_tiles[u] = wp.tile([128, QCH * HDV], F32, tag="osb",
                                           bufs=2, name=f"osb{u}")
                osb = osb_tiles[u]
                nc.vector.tensor_copy(
                    osb[:, qch * HDV:(qch + 1) * HDV], ctx[:, 0:HDV])
                if qch in (3, QCH - 1):
                    q0 = qc * 1024 + (0 if qch == 3 else 512)
                    o0 = 0 if qch == 3 else 4 * HDV
                    dst = outT[q0:q0 + 512, h * HDV:(h + 1) * HDV]
                    nc.sync.dma_start(
                        dst.rearrange("(a p) c -> p a c", p=128),
                        osb[:, o0:o0 + 4 * HDV].rearrange(
                            "p (a c) -> p a c", c=HDV))

            for u, (h, qc) in enumerate(units):
                m, off = divmod(h, 2)
                off *= HD
                kTh = kT_t[m][off:off + HD, :]
                qTh = qT_t[m][off:off + HD, :]
                q0 = qc * 1024
                for kc in range(KC):
                    ks = slice(kc * 128, (kc + 1) * 128)
                    psS = psp.tile([128, 1024], F32, tag="psS", bufs=3,
                                   name=f"psS{u}_{kc}")
                    for qq in range(2):
                        nc.tensor.matmul(
                            psS[:, qq * 512:(qq + 1) * 512],
                            kTh[:, ks],
                            qTh[:, q0 + qq * 512:q0 + (qq + 1) * 512],
                            start=True, stop=True)
                    expS = wp.tile([128, 1024], BF16, tag="expS", bufs=32,
                                   name=f"expS{u}_{kc}")
                    exp_tiles[(u, kc)] = expS
                    if kc in dve_kcs(u):
                        t1 = wp.tile([128, 1024], BF16, tag="tk1", bufs=4,
                                     name=f"tk1_{u}_{kc}")
                        t2 = wp.tile([128, 1024], BF16, tag="tk2", bufs=4,
                                     name=f"tk2_{u}_{kc}")
                        nc.vector.tensor_scalar(
                            t1[:].bitcast(U16), psS[:], AT,
                            dvb1[:, kc:kc + 1], op0=ALU.mult, op1=ALU.add)
                        nc.vector.tensor_scalar(
                            t2[:].bitcast(U16), t1[:].bitcast(U16), 64, None,
                            op0=ALU.subtract)
                        nc.vector.tensor_tensor(
                            expS[:], t1[:], t2[:], op=ALU.add)
                    else:
                        nc.scalar.activation(expS[:], psS[:], AFT.Exp,
                                             bias=actb[:, kc:kc + 1],
                                             scale=0.125)
                    # interleave: previous unit's ctx chains (one per 2 kc)
                    if u > 0 and kc % 2 == 1:
                        emit_ctx_chain(u - 1, kc // 2)
                    # deferred projection drains (kc%3 cadence: ~6/unit,
                    # lasts through unit 8 to keep PE fed in late units)
                    if u == 0:
                        proj_v(kc, 0, 3)
                        if kc in (4, 5):
                            proj_one(qT_t, wqall, bq_sb, 0, kc - 4 + 2)
                        elif kc in (6, 7):
                            proj_one(kT_t, wkall, bk_sb, 0, kc - 6 + 2)
                    elif kc % 3 == 0:
                        drain_one()
                        if u >= 7 and pending and kc == 9:
                            drain_one()
                if u == len(units) - 1:
                    for qch in range(QCH):
                        emit_ctx_chain(u, qch)

    nc.compile()
    return nc


def _get_nc():
    global _CACHED
    if _CACHED is None:
        _CACHED = _build()
    return _CACHED


def kernel(hidden_states, attention_mask, Wq, bq, Wk, bk, Wv, bv):
    hidden_states = np.asarray(hidden_states, np.float32)
    attention_mask = np.asarray(attention_mask, np.float32)
    Wq, Wk, Wv = (np.asarray(w, np.float32) for w in (Wq, Wk, Wv))
    bq, bk, bv = (np.asarray(b, np.float32) for b in (bq, bk, bv))

    nc = _get_nc()
    in_maps = []
    for core in range(N_CORES):
        b, g = divmod(core, 2)
        cs = slice(g * DL, (g + 1) * DL)
        xTb = hidden_states[b].T.reshape(KT, 128, S).transpose(1, 0, 2)
        xTb = np.ascontiguousarray(xTb).reshape(128, KT * S).astype(
            ml_dtypes.bfloat16)
        mk = np.ascontiguousarray(
            attention_mask[b, 0, 0, :].reshape(KC, 128).T).astype(np.float32)
        bp = np.concatenate([
            mk - 2.0 + LNS,
            AE * mk + TB1,
            bq[cs].reshape(M3, 128).T,
            bk[cs].reshape(M3, 128).T,
        ], axis=1).astype(np.float32)
        def mmaj(W):   # [768, 384] -> [128, (m*6+k)*128] m-major image
            a = W[cs, :].T.reshape(KT, 128, M3, 128).transpose(1, 2, 0, 3)
            return np.ascontiguousarray(a).reshape(
                128, KT * DL).astype(ml_dtypes.bfloat16)

        def kmaj(W):   # [768, 384] -> [128, k*384] k-major image
            a = W[cs, :].T.reshape(KT, 128, DL).transpose(1, 0, 2)
            return np.ascontiguousarray(a).reshape(
                128, KT * DL).astype(ml_dtypes.bfloat16)

        in_maps.append({
            "xH": xTb,
            "wqH": mmaj(Wq),
            "wkH": mmaj(Wk),
            "wvH": kmaj(Wv),
            "bpackT": np.ascontiguousarray(bp),
        })

    res = run_bass_kernel_spmd(nc, in_maps, core_ids=list(range(N_CORES)))

    out = np.empty((B, S, H), np.float32)
    for core in range(N_CORES):
        b, g = divmod(core, 2)
        oT = res.results[core]["outT"]              # [2048, 6*65]
        oT = oT.reshape(S, NH_LOC, HDV)
        ctx = oT[:, :, :HD] / oT[:, :, HD:HDV]      # [2048, 6, 64]
        cols = slice(g * DL, (g + 1) * DL)
        out[b, :, cols] = ctx.reshape(S, DL) + bv[cols][None, :]
    return out


# revision 12
# speedup vs baseline: 1.0624x; 1.0491x over previous
"""BertSelfAttention on 8 TRN2 NeuronCores (Bass/Tile).

Sharding: core = (b, g) for b in 0..3 (batch), g in 0..1 (head group of 6
heads = 384 cols of the QKV projections). Pure SPMD, no collectives.

Per-core device kernel (all bf16 storage, f32 PSUM):
  xT  [768, 2048]  = hidden_states[b].T          (host pre-transpose)
  wq/wk/wv [768, 384] = W[g-slice].T             (kxm layout)
  QT/KT = w.T @ xT -> [384, 2048] (d on partitions) + bias via DVE
  V = xT.T @ wv -> [2048, 384] (s on partitions), stored per 128-row chunk
  as [128, 6, 65] with a ones column per head (softmax denominator).

  Attention per (head, 1024-q block) "unit":
   phase 1, per kc (16 key chunks of 128):
     scores^T [128k, 1024q] = KT_h[:, kc].T @ QT_h on PE (2 matmuls)
     expS = exp(scores/8 + mask - 2) -> bf16 SBUF, on EITHER:
       - ACT: native Exp (scale=0.125, per-kc bias AP = mask-2), or
       - DVE (offload, tunable subset): two Schraudolph bf16 bit-tricks
         offset by half an octave, combined e = t2*sqrt(2) + t1 via
         scalar_tensor_tensor (max rel err ~0.9%). uint16 convert on DVE
         saturates+rounds-to-nearest (verified on HW).
     The -2 shift cancels between numerator and denominator.
   phase 2 (emitted interleaved into the NEXT unit's phase 1 so the
   in-order PE queue never serializes behind ACT), per qch (8 q chunks
   of 128):
     ctx[128q, 65] = sum_kc expS_kc[:, qch].T @ V_kc[:, h]  (16 matmuls,
     free dim 65 -> 27ns each; "flipped" orientation cuts ctx PE time 2x
     vs ctx^T since all 128 output partitions are used)
     row sums land in col 64 via the V ones column; DVE copies to SBUF,
     DMA to outT[q, h*65:...].

  The V projection and the m>=1 Q/K projection chunks are deferred and
  drained one item per (kc) slot into the attention stream to fill PE
  while ACT/DVE grind exps.

Host divides ctx cols 0..63 by col 64 (denominator), adds bv.
"""

import sys

for _p in ("/opt/trn_rl_repo",):
    if _p not in sys.path:
        sys.path.insert(0, _p)

import numpy as np
import ml_dtypes

import concourse.bass as bass  # noqa: F401
import concourse.mybir as mybir
from concourse import bacc, tile
from concourse.bass_utils import run_bass_kernel_spmd

AFT = mybir.ActivationFunctionType
ALU = mybir.AluOpType
BF16 = mybir.dt.bfloat16
F32 = mybir.dt.float32
U16 = mybir.dt.uint16

B, S, H = 4, 2048, 768
NH, HD = 12, 64
N_CORES = 8
NH_LOC = 6          # heads per core
DL = NH_LOC * HD    # 384 local projection cols
KT = H // 128       # 6 k-tiles over the hidden dim
M3 = DL // 128      # 3 m-chunks of the local projections
KC = S // 128       # 16 key chunks
QC = S // 1024      # 2 query blocks of 1024 per head
QCH = 8             # 128-row q chunks per block
HDV = HD + 1        # per-head V cols incl. ones column

# exp bit-trick constants: bf16 Schraudolph sample t1 = u16(scores*AT
# + bias); second sample half an octave lower derived by u16 subtract
# (4x DVE mode); combined with a bf16 add (2x mode): e = t1 + (t1-64
# bytes) = 0.85355*exp(arg) (constant scale; ACT chunks are scaled to
# match via LNS in their bias so the softmax ratio is unaffected).
AT = 23.083120625
TB1 = 15752.170070
AE = 184.664965     # byte16 per unit of exp argument (for mask folding)
LNS = -0.15834718   # ln((1+2**-0.5)/2)

# which (unit, kc) chunks run exp on DVE instead of ACT: none in the
# PE-bound unit 0; more in late units once V/QK copies are done and DVE
# has slack.
def dve_kcs(u):
    if u == 0:
        return ()
    if u <= 2:
        return (6, 14)
    if u >= 8:
        return (2, 5, 8, 11, 14)
    return (2, 6, 10, 14)

_CACHED = None


def _build():
    nc = bacc.Bacc("TRN2", target_bir_lowering=False, debug=False,
                   num_devices=N_CORES)
    # inputs are host-prearranged images of the SBUF tiles:
    # xH [128, k*2048]; wq/wk m-major [128, (m*6+k)*128]; wv k-major
    # [128, k*384]
    xH = nc.dram_tensor("xH", [128, KT * S], BF16, kind="ExternalInput").ap()
    wqH = nc.dram_tensor("wqH", [128, KT * DL], BF16,
                         kind="ExternalInput").ap()
    wkH = nc.dram_tensor("wkH", [128, KT * DL], BF16,
                         kind="ExternalInput").ap()
    wvH = nc.dram_tensor("wvH", [128, KT * DL], BF16,
                         kind="ExternalInput").ap()
    # packed small inputs: actb 0:16, dvb1 16:32, bq 32:35, bk 35:38
    bpackT = nc.dram_tensor("bpackT", [128, 2 * KC + 2 * M3], F32,
                            kind="ExternalInput").ap()
    outT = nc.dram_tensor("outT", [S, NH_LOC * HDV], F32,
                          kind="ExternalOutput").ap()

    with tile.TileContext(nc) as tc:
        with (
            tc.tile_pool(name="persist", bufs=1) as pp,
            tc.tile_pool(name="work", bufs=1) as wp,
            tc.tile_pool(name="psum", bufs=1, space="PSUM") as psp,
        ):
            # ---- persistent SBUF tensors (k-tiles packed side by side
            # in one tile each so input DMAs are few and large) ----
            xall = pp.tile([128, KT * S], BF16, tag="xall", name="xall")
            wqall = pp.tile([128, KT * DL], BF16, tag="wqall", name="wqall")
            wkall = pp.tile([128, KT * DL], BF16, tag="wkall", name="wkall")
            wvall = pp.tile([128, KT * DL], BF16, tag="wvall", name="wvall")
            x_t = [xall[:, k * S:(k + 1) * S] for k in range(KT)]
            # wq/wk are m-major: slice (m, k) at (m*KT+k)*128
            wv_t = [wvall[:, k * DL:(k + 1) * DL] for k in range(KT)]

            def wqk(wall, k, m):
                c = (m * KT + k) * 128
                return wall[:, c:c + 128]
            qT_t = [pp.tile([128, S], BF16, tag=f"qT{m}", name=f"qT{m}")
                    for m in range(M3)]
            kT_t = [pp.tile([128, S], BF16, tag=f"kT{m}", name=f"kT{m}")
                    for m in range(M3)]
            v_t = [pp.tile([128, NH_LOC * HDV], BF16, tag=f"v{s}",
                           name=f"v{s}") for s in range(KC)]
            bpack = pp.tile([128, 2 * KC + 2 * M3], F32, tag="bpack",
                            name="bpack")
            actb = bpack[:, 0:KC]
            dvb1 = bpack[:, KC:2 * KC]
            bq_sb = bpack[:, 2 * KC:2 * KC + M3]
            bk_sb = bpack[:, 2 * KC + M3:2 * KC + 2 * M3]
            ones3 = pp.tile([128, 3], BF16, tag="ones3", name="ones3")
            nc.vector.memset(ones3[:], 1.0)

            # ---- DMA inputs: sources mirror SBUF layouts, so every DMA
            # is a contiguous block copy; ordered so the m=0 projection and
            # unit-0 attention start earliest.
            xv = xall.rearrange("p (k c) -> p k c", c=S)
            xsv = xH.rearrange("p (k c) -> p k c", c=S)
            nc.sync.dma_start(xv[:, :, 0:256], xsv[:, :, 0:256])
            nc.sync.dma_start(wqall[:, 0:6 * 128], wqH[:, 0:6 * 128])
            nc.sync.dma_start(wkall[:, 0:6 * 128], wkH[:, 0:6 * 128])
            nc.sync.dma_start(xv[:, :, 256:512], xsv[:, :, 256:512])
            nc.sync.dma_start(bpack[:], bpackT[:])
            nc.sync.dma_start(xv[:, :, 512:S], xsv[:, :, 512:S])
            nc.sync.dma_start(wvall[:], wvH[:])
            nc.sync.dma_start(wqall[:, 6 * 128:], wqH[:, 6 * 128:])
            nc.sync.dma_start(wkall[:, 6 * 128:], wkH[:, 6 * 128:])

            def proj_one(dst, w_t, b_sb, m, n, width=512):  # w_t: wall AP
                """Project one s-chunk of QT[m] or KT[m]."""
                ns = slice(n * width, (n + 1) * width)
                ps = psp.tile([128, width], F32, tag="psS", bufs=3,
                              name=f"psp{dst[m].name}_{n}_{width}")
                for k in range(KT):
                    nc.tensor.matmul(
                        ps[:],
                        wqk(w_t, k, m),
                        x_t[k][:, ns],
                        start=(k == 0), stop=(k == KT - 1))
                nc.vector.tensor_scalar_add(
                    dst[m][:, ns], ps[:], b_sb[:, m:m + 1])

            def proj_v(sc, ha, hb):
                """Project heads [ha, hb) of one 128-row s-chunk of V."""
                nh = hb - ha
                ps = psp.tile([128, nh * HD], F32, tag="psS", bufs=3,
                              name=f"psv{sc}_{ha}")
                for k in range(KT):
                    nc.tensor.matmul(
                        ps[:],
                        x_t[k][:, sc * 128:(sc + 1) * 128],
                        wv_t[k][:, ha * HD:hb * HD],
                        start=(k == 0), stop=(k == KT - 1))
                v3 = v_t[sc].rearrange("p (h e) -> p h e", e=HDV)
                nc.vector.tensor_copy(
                    v3[:, ha:hb, 0:HD],
                    ps[:].rearrange("p (h e) -> p h e", e=HD))
                nc.gpsimd.tensor_copy(v3[:, ha:hb, HD:HDV], ones3[:, 0:nh])

            # Deferred projection work, drained into the attention stream.
            # Deadlines (strict-before): m1 before unit 4 phase1, V heads 3-6
            # before unit 6 phase2 (emitted during unit 7), m2 before unit 8.
            pending = []
            for n in range(8):
                pending.append(("k", 1, n))
                pending.append(("q", 1, n))
            pending += [("v", sc, 3, 6) for sc in range(KC)]
            for n in range(8):
                pending.append(("k", 2, n))
                pending.append(("q", 2, n))

            def drain_one():
                if pending:
                    item = pending.pop(0)
                    if item[0] == "v":
                        proj_v(item[1], item[2], item[3])
                    elif item[0] == "q":
                        proj_one(qT_t, wqall, bq_sb, item[1], item[2], 256)
                    else:
                        proj_one(kT_t, wkall, bk_sb, item[1], item[2], 256)

            # m=0 Q/K projections for q-columns 0-1024 up front (first
            # chunks 256-wide so PE starts as soon as the first x DMA
            # lands); n=2,3 are emitted early in unit 0.
            proj_one(qT_t, wqall, bq_sb, 0, 0, 256)
            proj_one(kT_t, wkall, bk_sb, 0, 0, 256)
            proj_one(qT_t, wqall, bq_sb, 0, 1, 256)
            proj_one(kT_t, wkall, bk_sb, 0, 1, 256)
            proj_one(qT_t, wqall, bq_sb, 0, 2, 256)
            proj_one(kT_t, wkall, bk_sb, 0, 2, 256)
            proj_one(qT_t, wqall, bq_sb, 0, 3, 256)
            proj_one(kT_t, wkall, bk_sb, 0, 3, 256)

            # ---- attention units ----
            units = [(h, qc) for h in range(NH_LOC) for qc in range(QC)]
            exp_tiles = {}

            osb_tiles = {}

            def emit_ctx_chain(u, qch):
                """ctx[128q, 65] for unit u, q chunk qch (flip orientation),
                then DVE copy into the unit's staging tile; one DMA per
                unit after the last chain (HWDGE overhead is per-DMA)."""
                h, qc = units[u]
                ctx = psp.tile([128, 512], F32, tag="ctx", bufs=2,
                               name=f"ctx{u}_{qch}")
                qs = slice(qch * 128, (qch + 1) * 128)
                for kc in range(KC):
                    nc.tensor.matmul(
                        ctx[:, 0:HDV],
                        exp_tiles[(u, kc)][:, qs],
                        v_t[kc][:, h * HDV:(h + 1) * HDV],
                        start=(kc == 0), stop=(kc == KC - 1))
                if qch == 0:
                    osb_tiles[u] = wp.tile([128, QCH * HDV], F32, tag="osb",
                                           bufs=2, name=f"osb{u}")
                osb = osb_tiles[u]
                nc.vector.tensor_copy(
                    osb[:, qch * HDV:(qch + 1) * HDV], ctx[:, 0:HDV])
                if qch in (3, QCH - 1):
                    q0 = qc * 1024 + (0 if qch == 3 else 512)
                    o0 = 0 if qch == 3 else 4 * HDV
                    dst = outT[q0:q0 + 512, h * HDV:(h + 1) * HDV]
                    nc.sync.dma_start(
                        dst.rearrange("(a p) c -> p a c", p=128),
                        osb[:, o0:o0 + 4 * HDV].rearrange(
                            "p (a c) -> p a c", c=HDV))

            for u, (h, qc) in enumerate(units):
                m, off = divmod(h, 2)
                off *= HD
                kTh = kT_t[m][off:off + HD, :]
                qTh = qT_t[m][off:off + HD, :]
                q0 = qc * 1024
                for kc in range(KC):
                    ks = slice(kc * 128, (kc + 1) * 128)
                    psS = psp.tile([128, 1024], F32, tag="psS", bufs=3,
                                   name=f"psS{u}_{kc}")
                    for qq in range(2):
                        nc.tensor.matmul(
                            psS[:, qq * 512:(qq + 1) * 512],
                            kTh[:, ks],
                            qTh[:, q0 + qq * 512:q0 + (qq + 1) * 512],
                            start=True, stop=True)
                    expS = wp.tile([128, 1024], BF16, tag="expS", bufs=32,
                                   name=f"expS{u}_{kc}")
                    exp_tiles[(u, kc)] = expS
                    if kc in dve_kcs(u):
                        t1 = wp.tile([128, 1024], BF16, tag="tk1", bufs=4,
                                     name=f"tk1_{u}_{kc}")
                        t2 = wp.tile([128, 1024], BF16, tag="tk2", bufs=4,
                                     name=f"tk2_{u}_{kc}")
                        nc.vector.tensor_scalar(
                            t1[:].bitcast(U16), psS[:], AT,
                            dvb1[:, kc:kc + 1], op0=ALU.mult, op1=ALU.add)
                        nc.vector.tensor_scalar(
                            t2[:].bitcast(U16), t1[:].bitcast(U16), 64, None,
                            op0=ALU.subtract)
                        nc.vector.tensor_tensor(
                            expS[:], t1[:], t2[:], op=ALU.add)
                    else:
                        nc.scalar.activation(expS[:], psS[:], AFT.Exp,
                                             bias=actb[:, kc:kc + 1],
                                             scale=0.125)
                    # interleave: previous unit's ctx chains (one per 2 kc)
                    if u > 0 and kc % 2 == 1:
                        emit_ctx_chain(u - 1, kc // 2)
                    # deferred projection drains (kc%3 cadence: ~6/unit,
                    # lasts through unit 8 to keep PE fed in late units)
                    if u == 0:
                        proj_v(kc, 0, 3)
                        if kc in (4, 5):
                            proj_one(qT_t, wqall, bq_sb, 0, kc - 4 + 2)
                        elif kc in (6, 7):
                            proj_one(kT_t, wkall, bk_sb, 0, kc - 6 + 2)
                    elif kc % 2 == 0:
                        drain_one()
                if u == len(units) - 1:
                    for qch in range(QCH):
                        emit_ctx_chain(u, qch)

    nc.compile()
    return nc


def _get_nc():
    global _CACHED
    if _CACHED is None:
        _CACHED = _build()
    return _CACHED


def kernel(hidden_states, attention_mask, Wq, bq, Wk, bk, Wv, bv):
    hidden_states = np.asarray(hidden_states, np.float32)
    attention_mask = np.asarray(attention_mask, np.float32)
    Wq, Wk, Wv = (np.asarray(w, np.float32) for w in (Wq, Wk, Wv))
    bq, bk, bv = (np.asarray(b, np.float32) for b in (bq, bk, bv))

    nc = _get_nc()
    in_maps = []
    for core in range(N_CORES):
        b, g = divmod(core, 2)
        cs = slice(g * DL, (g + 1) * DL)
        xTb = hidden_states[b].T.reshape(KT, 128, S).transpose(1, 0, 2)
        xTb = np.ascontiguousarray(xTb).reshape(128, KT * S).astype(
            ml_dtypes.bfloat16)
        mk = np.ascontiguousarray(
            attention_mask[b, 0, 0, :].reshape(KC, 128).T).astype(np.float32)
        bp = np.concatenate([
            mk - 2.0 + LNS,
            AE * mk + TB1,
            bq[cs].reshape(M3, 128).T,
            bk[cs].reshape(M3, 128).T,
        ], axis=1).astype(np.float32)
        def mmaj(W):   # [768, 384] -> [128, (m*6+k)*128] m-major image
            a = W[cs, :].T.reshape(KT, 128, M3, 128).transpose(1, 2, 0, 3)
            return np.ascontiguousarray(a).reshape(
                128, KT * DL).astype(ml_dtypes.bfloat16)

        def kmaj(W):   # [768, 384] -> [128, k*384] k-major image
            a = W[cs, :].T.reshape(KT, 128, DL).transpose(1, 0, 2)
            return np.ascontiguousarray(a).reshape(
                128, KT * DL).astype(ml_dtypes.bfloat16)

        in_maps.append({
            "xH": xTb,
            "wqH": mmaj(Wq),
            "wkH": mmaj(Wk),
            "wvH": kmaj(Wv),
            "bpackT": np.ascontiguousarray(bp),
        })

    res = run_bass_kernel_spmd(nc, in_maps, core_ids=list(range(N_CORES)))

    out = np.empty((B, S, H), np.float32)
    for core in range(N_CORES):
        b, g = divmod(core, 2)
        oT = res.results[core]["outT"]              # [2048, 6*65]
        oT = oT.reshape(S, NH_LOC, HDV)
        ctx = oT[:, :, :HD] / oT[:, :, HD:HDV]      # [2048, 6, 64]
        cols = slice(g * DL, (g + 1) * DL)
        out[b, :, cols] = ctx.reshape(S, DL) + bv[cols][None, :]
    return out
